# revision 16
# baseline (speedup 1.0000x reference)
"""SphericalConv (gather-based 3x3 conv + 2x nearest upsample) on 8 trn2 cores.

Strategy (data-parallel over batch, one batch image per core):
  1. Feature is uploaded as fp16 (host cast) -> halves the dominant HBM read,
     and fp16 matmuls run 4x faster on PE than fp32.
  2. S_k = sum_c w[c,k] * F[c,:,:] for the 9 taps via PE matmuls with a
     block-diagonal stationary [128, 32] fp16 (row quad j, taps k in columns
     9j+k).  Streamed over 256 source rows, 8 rows per ft tile; ft loads are
     round-robined over the three DMA-capable engines (SP / ACT / Pool).
  3. S rows land in PSUM, are cast to fp16 into a [128, 4, 512] staging tile
     (partition = PE output slot, incl. zero pads), and flushed to a DRAM
     scratch with one 128-partition DMA per 32 source rows.  A single pair of
     row-parallel DRAM->DRAM copies then doubles every row ([row|row]) so a
     circular shift is one contiguous 512-elem read.
  4. The spherical gather out[h,w] = sum_k S_k[gi(h,k), (w+d(h,k)) mod W] runs
     as 3 full-width indirect DMA gathers (3 taps x 2 row-parities each);
     offsets are host-computed from gi/gj.
  5. 9-tap sum on DVE (fp16), column-double + f32 cast on DVE+ACT, one
     output DMA of the 256 distinct upsampled rows, and one row-parallel
     DRAM->DRAM copy to duplicate odd rows.

The gi/gj maps produced by the gnomonic projection are row-structured
(gi constant along w; gj a per-row circular shift).  Verified on the host;
arbitrary index maps fall back to a host computation.
"""

import sys

sys.path.insert(0, "/opt/trn_rl_repo")

import numpy as np

B, C, H, W = 8, 64, 256, 512
NCORES = 8
TAPS = 9
NROWS = 4096  # 8 fl x 4 q x 128 slots (incl. pad slots)
ROWLEN = 1024  # doubled S row (fp16 elements)

_prog_cache = {}


def _split_multi_waits(nc, mybir):
    # This container's walrus rejects >1 sync wait per instruction; hoist the
    # extra waits onto standalone event-semaphore instructions just before.
    n = 0
    for blk in nc.m.functions[0].blocks:
        insts = blk.instructions
        new, changed = [], False
        for i in insts:
            si = i.sync_info
            if si is not None and len(si.on_wait) > 1:
                waits = list(si.on_wait)
                for w in waits[:-1]:
                    n += 1
                    ev = mybir.InstEventSemaphore(
                        name=f"wsplit_{n}_{i.name}",
                        engine=i.engine,
                        sync_info=mybir.SyncInfo(on_wait=[w], on_update=[]),
                    )
                    new.append(ev)
                i.sync_info = mybir.SyncInfo(
                    on_wait=[waits[-1]], on_update=list(si.on_update)
                )
                changed = True
            new.append(i)
        if changed:
            blk.instructions = new


def _build_program(split_waits=True):
    key = "nc" if split_waits else "nc_raw"
    if key in _prog_cache:
        return _prog_cache[key]

    import concourse.bass as bass
    import concourse.tile as tile
    from concourse import mybir
    from concourse.bass import AP, IndirectOffsetOnAxis

    f16 = mybir.dt.float16
    f32 = mybir.dt.float32

    nc = bass.Bass("TRN2", target_bir_lowering=False, debug=False)
    feat = nc.dram_tensor("feat16", [C, H, W], f16, kind="ExternalInput")
    wbd = nc.dram_tensor("wbd", [128, 32], f16, kind="ExternalInput")
    offs = nc.dram_tensor("offs", [128, TAPS, 2], mybir.dt.int32, kind="ExternalInput")
    out = nc.dram_tensor("out", [2 * H, 2 * W], f32, kind="ExternalOutput")
    s2s = nc.dram_tensor("s2s", [NROWS * 512], f16)  # single-width scratch
    s2x = nc.dram_tensor("s2x", [NROWS * ROWLEN], f16)  # doubled scratch

    # ft loads round-robined over the three DMA-capable engines (ACT first
    # so group 0 isn't stuck behind the const loads on SP)
    FT_ENG = ["scalar", "gpsimd", "sync"]
    FLUSH_ENG = ["sync", "scalar"]
    FLUSH_LAG = 2  # flush f emitted after group 4f+3+4*LAG (waits are stale)

    def _dbl_rows(nc, AP, s2s, s2x, r0, nr, eng):
        # row-parallel DRAM->DRAM doubling of s2s rows [r0, r0+nr)
        for dbl in range(2):
            dst = AP(s2x, r0 * ROWLEN + dbl * 512, [(ROWLEN, nr), (1, 512)])
            getattr(nc, eng).dma_start(
                dst, AP(s2s, r0 * 512, [(512, nr), (1, 512)])
            )

    with tile.TileContext(nc) as tc:
        with (
            tc.tile_pool(name="consts", bufs=1) as consts,
            tc.tile_pool(name="ft", bufs=6) as ftp,
            tc.tile_pool(name="ps", bufs=8, space="PSUM") as psp,
            tc.tile_pool(name="stage", bufs=3) as stp,
            tc.tile_pool(name="gath", bufs=1) as gap,
            tc.tile_pool(name="outp", bufs=1) as outp,
        ):
            wt = consts.tile([128, 32], f16)
            nc.sync.dma_start(wt[:], wbd.ap())
            offs_t = consts.tile([128, TAPS, 2], mybir.dt.int32)
            nc.sync.dma_start(offs_t[:], offs.ap())

            def emit_flush(fl, sts, q0=0, nq=4, eng=None):
                # 128-partition DMA: s2s row = (fl*4 + q)*128 + slot
                st = sts[fl]
                dst = AP(
                    s2s,
                    (fl * 4 + q0) * 128 * 512,
                    [(512, 128), (128 * 512, nq), (1, 512)],
                )
                if nq == 1:
                    dst = AP(s2s, (fl * 4 + q0) * 128 * 512, [(512, 128), (1, 512)])
                getattr(nc, eng or FLUSH_ENG[fl % 2]).dma_start(
                    dst, st[:, q0 : q0 + nq, :] if nq < 4 else st[:]
                )

            # main loop: 32 groups x 8 contiguous rows (8m + 4j + i)
            sts = {}
            for m in range(32):
                ft = ftp.tile([128, 2048], f16)
                # partition p = 64j + c (j = row quad), free = (i, col):
                # element = feat[c, 8m + 4j + i, col]  (4 contiguous rows/half)
                src = AP(feat, 8 * m * W, [(4 * W, 2), (H * W, C), (1, 4 * W)])
                if m == 0:
                    # split the first load across two engines so matmul 0
                    # starts ~1us earlier
                    srcA = AP(feat, 0, [(4 * W, 2), (H * W, C), (1, 2 * W)])
                    srcB = AP(feat, 2 * W, [(4 * W, 2), (H * W, C), (1, 2 * W)])
                    nc.scalar.dma_start(ft[:, 0:1024], srcA)
                    nc.gpsimd.dma_start(ft[:, 1024:2048], srcB)
                else:
                    getattr(nc, FT_ENG[m % 3]).dma_start(ft[:], src)

                ps = psp.tile([128, 512], f32)
                for i in range(4):
                    nc.tensor.matmul(
                        ps[32 * i : 32 * i + 32, :],
                        lhsT=wt[:],
                        rhs=ft[:, 512 * i : 512 * (i + 1)],
                        start=True,
                        stop=True,
                        tile_position=(0, 32 * i),
                    )

                if m % 4 == 0:
                    sts[m // 4] = stp.tile([128, 4, 512], f16, name="st")
                q = m % 4
                # cast PSUM -> SBUF fp16; last groups alternate DVE/ACT so no
                # copy backlog trails the final ft load
                if m >= 29 and m % 2 == 1:
                    nc.scalar.copy(sts[m // 4][:, q, :], ps[:])
                else:
                    nc.vector.tensor_copy(sts[m // 4][:, q, :], ps[:])

                if m >= 4 * (1 + FLUSH_LAG) - 1 and (m - 4 * FLUSH_LAG) % 4 == 3:
                    emit_flush((m - 4 * FLUSH_LAG) // 4, sts)
                if m == 31:
                    # end-game: flush 6 whole, flush 7 split 3+1 so only the
                    # last 8-row group's tiny flush trails the final copy
                    emit_flush(6, sts, eng="sync")
                    _dbl_rows(nc, AP, s2s, s2x, 0, 7 * 512 - 128, "scalar")
                    emit_flush(7, sts, q0=0, nq=3, eng="sync")
                    _dbl_rows(nc, AP, s2s, s2x, 7 * 512 - 128, 512, "sync")
                    emit_flush(7, sts, q0=3, nq=1, eng="scalar")
                    _dbl_rows(nc, AP, s2s, s2x, 7 * 512 + 384, 128, "scalar")

            # slot-chunk gathers (taps {0-3}, {4,5}, {6,7}, {8}) with
            # incremental accumulation: after each gather only ~1 add of
            # catch-up work remains on DVE.  in_ shaped (NROWS*2, 512) with
            # axis=1 so offsets stay element-granular with a 1KB inner run.
            CHUNKS = [(0, 4), (4, 2), (6, 2), (8, 1)]
            gas = []
            for g, (k0, nk) in enumerate(CHUNKS):
                ga = gap.tile([128, nk, 2, 512], f16, name=f"gag{g}")
                nc.gpsimd.indirect_dma_start(
                    out=ga[:].opt(keep_dims={0}),
                    out_offset=None,
                    in_=AP(s2x, 0, [(512, NROWS * 2), (1, 512)]),
                    in_offset=IndirectOffsetOnAxis(
                        ap=offs_t[:, k0 : k0 + nk, :], axis=1
                    ),
                )
                gas.append(ga)

            o = outp.tile([128, 2, 512], f16)
            u = outp.tile([128, 2, 512], f16)
            # taps 0-3 (tree in gag0), then +(4+5), +(6+7), +8
            nc.vector.tensor_add(o[:], gas[0][:, 0], gas[0][:, 1])
            nc.vector.tensor_add(u[:], gas[0][:, 2], gas[0][:, 3])
            nc.vector.tensor_add(o[:], o[:], u[:])
            nc.vector.tensor_add(u[:], gas[1][:, 0], gas[1][:, 1])
            nc.vector.tensor_add(o[:], o[:], u[:])
            nc.vector.tensor_add(u[:], gas[2][:, 0], gas[2][:, 1])
            nc.vector.tensor_add(o[:], o[:], u[:])
            nc.vector.tensor_add(o[:], o[:], gas[3][:, 0])

            # column-double + cast to f32 (both on DVE; ACT is mid-DMA)
            o2 = outp.tile([128, 2, 512, 2], f32)
            nc.vector.tensor_copy(o2[:, :, :, 0], o[:])
            nc.vector.tensor_copy(o2[:, :, :, 1], o[:])

            # upsampled rows 4p + 2hd + a: even rows from SP, odd from ACT,
            # concurrently (both read o2).
            dst = AP(out, 0, [(4 * 2 * W, 128), (2 * 2 * W, 2), (1, 2 * W)])
            nc.sync.dma_start(dst, o2[:].opt(keep_dims={0}))
            dst2 = AP(out, 2 * W, [(4 * 2 * W, 128), (2 * 2 * W, 2), (1, 2 * W)])
            nc.scalar.dma_start(dst2, o2[:].opt(keep_dims={0}))

    if split_waits:
        _split_multi_waits(nc, mybir)
    _prog_cache[key] = nc
    return nc


def _structured(gi, gj):
    if not all(np.array_equal(gi[:, :, k], np.broadcast_to(gi[:, :1, k], (H, W))) for k in range(TAPS)):
        return False
    d = (gj - np.arange(W, dtype=np.int64)[None, :, None]) % W
    return all(np.array_equal(d[:, :, k], np.broadcast_to(d[:, :1, k], (H, W))) for k in range(TAPS))


def _host_fallback(feature, weight, gi, gj):
    # correct-but-slow path for arbitrary (non roll-structured) index maps
    wflat = weight.reshape(1, C, TAPS).astype(np.float32)
    outc = np.zeros((B, H, W), np.float32)
    for k in range(TAPS):
        xk = feature[:, :, gi[:, :, k], gj[:, :, k]]
        outc += np.einsum("bchw,c->bhw", xk, wflat[0, :, k])
    up = np.repeat(np.repeat(outc, 2, axis=1), 2, axis=2)
    return up[:, None].astype(np.float32)


def _make_device_inputs(weight, gi, gj):
    # block-diag stationary [128, 32]: wt[64*j + c, 9*j + k] = w[c,k]
    w9 = np.asarray(weight, np.float32).reshape(C, TAPS)
    wbd = np.zeros((128, 32), np.float16)
    for j in range(2):
        wbd[64 * j : 64 * j + 64, 9 * j : 9 * j + 9] = w9

    r = gi[:, 0, :].astype(np.int64)  # [H, 9]
    d = gj[:, 0, :].astype(np.int64) % W  # shift per (h, k)

    # scratch row id for source row r = 8m + 4j + i:
    # slot = 32i + 9j + k, row = m*128 + slot  (m = r//8)
    mm = r // 8
    rem = r % 8
    j2 = rem // 4
    i4 = rem % 4
    row_id = (mm * 128 + 32 * i4 + 9 * j2) + np.arange(TAPS)[None, :]
    off_hk = row_id * ROWLEN + d  # [H, 9]

    # offs[p, k, hd] for output row h = 2p + hd
    offs = np.zeros((128, TAPS, 2), np.int32)
    for hd in range(2):
        offs[:, :, hd] = off_hk[2 * np.arange(128) + hd, :]
    return wbd, offs


def _run_device(feature16, wbd, offs, trace=False, trace_kwargs=None):
    from concourse.bass_utils import run_bass_kernel_spmd

    nc = _build_program()
    in_maps = [
        {"feat16": np.ascontiguousarray(feature16[b]), "wbd": wbd, "offs": offs}
        for b in range(B)
    ]
    kw = {}
    if trace:
        kw["trace"] = True
        if trace_kwargs:
            kw.update(trace_kwargs)
    return run_bass_kernel_spmd(nc, in_maps, list(range(NCORES)), **kw)


def kernel(feature, weight, gi, gj):
    feature = np.asarray(feature, dtype=np.float32)
    weight = np.asarray(weight, dtype=np.float32)
    gi = np.asarray(gi)
    gj = np.asarray(gj)

    if not _structured(gi, gj):
        return _host_fallback(feature, weight, gi, gj)

    wbd, offs = _make_device_inputs(weight, gi, gj)
    feature16 = feature.astype(np.float16)
    res = _run_device(feature16, wbd, offs)
    out = np.stack([res.results[b]["out"] for b in range(B)])
    return out[:, None].astype(np.float32)


# revision 17
# speedup vs baseline: 1.0190x; 1.0190x over previous
"""SphericalConv (gather-based 3x3 conv + 2x nearest upsample) on 8 trn2 cores.

Strategy (data-parallel over batch, one batch image per core):
  1. Feature is uploaded as fp16 (host cast) -> halves the dominant HBM read,
     and fp16 matmuls run 4x faster on PE than fp32.
  2. S_k = sum_c w[c,k] * F[c,:,:] for the 9 taps via PE matmuls with a
     block-diagonal stationary [128, 32] fp16 (row quad j, taps k in columns
     9j+k).  Streamed over 256 source rows, 8 rows per ft tile; ft loads are
     round-robined over the three DMA-capable engines (SP / ACT / Pool).
  3. S rows land in PSUM, are cast to fp16 into a [128, 4, 512] staging tile
     (partition = PE output slot, incl. zero pads), and flushed to a DRAM
     scratch with one 128-partition DMA per 32 source rows.  A single pair of
     row-parallel DRAM->DRAM copies then doubles every row ([row|row]) so a
     circular shift is one contiguous 512-elem read.
  4. The spherical gather out[h,w] = sum_k S_k[gi(h,k), (w+d(h,k)) mod W] runs
     as 3 full-width indirect DMA gathers (3 taps x 2 row-parities each);
     offsets are host-computed from gi/gj.
  5. 9-tap sum on DVE (fp16), column-double + f32 cast on DVE+ACT, one
     output DMA of the 256 distinct upsampled rows, and one row-parallel
     DRAM->DRAM copy to duplicate odd rows.

The gi/gj maps produced by the gnomonic projection are row-structured
(gi constant along w; gj a per-row circular shift).  Verified on the host;
arbitrary index maps fall back to a host computation.
"""

import sys

sys.path.insert(0, "/opt/trn_rl_repo")

import numpy as np

B, C, H, W = 8, 64, 256, 512
NCORES = 8
TAPS = 9
NROWS = 4096  # 8 fl x 4 q x 128 slots (incl. pad slots)
ROWLEN = 1024  # doubled S row (fp16 elements)
# gather tap groups with the max source row each may touch (host-verified):
# group A = taps {3,5,6,7,8} (rows <= 223), B = {0,2} (<= 231), C = {1,4}
TAP_PERM = (3, 5, 6, 7, 8, 0, 2, 1, 4)
TAP_GROUPS = ((0, 5, 3584), (5, 2, 3712), (7, 2, NROWS))  # (idx0, n, row limit)
TAP_MAXROW = (223, 231, 255)

_prog_cache = {}


def _split_multi_waits(nc, mybir):
    # This container's walrus rejects >1 sync wait per instruction; hoist the
    # extra waits onto standalone event-semaphore instructions just before.
    n = 0
    for blk in nc.m.functions[0].blocks:
        insts = blk.instructions
        new, changed = [], False
        for i in insts:
            si = i.sync_info
            if si is not None and len(si.on_wait) > 1:
                waits = list(si.on_wait)
                for w in waits[:-1]:
                    n += 1
                    ev = mybir.InstEventSemaphore(
                        name=f"wsplit_{n}_{i.name}",
                        engine=i.engine,
                        sync_info=mybir.SyncInfo(on_wait=[w], on_update=[]),
                    )
                    new.append(ev)
                i.sync_info = mybir.SyncInfo(
                    on_wait=[waits[-1]], on_update=list(si.on_update)
                )
                changed = True
            new.append(i)
        if changed:
            blk.instructions = new


def _build_program(split_waits=True):
    key = "nc" if split_waits else "nc_raw"
    if key in _prog_cache:
        return _prog_cache[key]

    import concourse.bass as bass
    import concourse.tile as tile
    from concourse import mybir
    from concourse.bass import AP, IndirectOffsetOnAxis

    f16 = mybir.dt.float16
    f32 = mybir.dt.float32

    nc = bass.Bass("TRN2", target_bir_lowering=False, debug=False)
    feat = nc.dram_tensor("feat16", [C, H, W], f16, kind="ExternalInput")
    wbd = nc.dram_tensor("wbd", [128, 32], f16, kind="ExternalInput")
    offs = nc.dram_tensor("offs", [128, TAPS, 2], mybir.dt.int32, kind="ExternalInput")
    out = nc.dram_tensor("out", [2 * H, 2 * W], f32, kind="ExternalOutput")
    s2s = nc.dram_tensor("s2s", [NROWS * 512], f16)  # single-width scratch
    s2x = nc.dram_tensor("s2x", [NROWS * ROWLEN], f16)  # doubled scratch

    # ft loads round-robined over the three DMA-capable engines (ACT first
    # so group 0 isn't stuck behind the const loads on SP)
    FT_ENG = ["scalar", "gpsimd", "sync"]
    FLUSH_ENG = ["sync", "scalar"]
    FLUSH_LAG = 2  # flush f emitted after group 4f+3+4*LAG (waits are stale)

    def _dbl_rows(nc, AP, s2s, s2x, r0, nr, eng):
        # row-parallel DRAM->DRAM doubling of s2s rows [r0, r0+nr)
        for dbl in range(2):
            dst = AP(s2x, r0 * ROWLEN + dbl * 512, [(ROWLEN, nr), (1, 512)])
            getattr(nc, eng).dma_start(
                dst, AP(s2s, r0 * 512, [(512, nr), (1, 512)])
            )

    with tile.TileContext(nc) as tc:
        with (
            tc.tile_pool(name="consts", bufs=1) as consts,
            tc.tile_pool(name="ft", bufs=6) as ftp,
            tc.tile_pool(name="ps", bufs=8, space="PSUM") as psp,
            tc.tile_pool(name="stage", bufs=3) as stp,
            tc.tile_pool(name="gath", bufs=1) as gap,
            tc.tile_pool(name="outp", bufs=1) as outp,
        ):
            wt = consts.tile([128, 32], f16)
            nc.sync.dma_start(wt[:], wbd.ap())
            offs_t = consts.tile([128, TAPS, 2], mybir.dt.int32)
            nc.sync.dma_start(offs_t[:], offs.ap())

            def emit_flush(fl, sts, q0=0, nq=4, eng=None):
                # 128-partition DMA: s2s row = (fl*4 + q)*128 + slot
                st = sts[fl]
                dst = AP(
                    s2s,
                    (fl * 4 + q0) * 128 * 512,
                    [(512, 128), (128 * 512, nq), (1, 512)],
                )
                if nq == 1:
                    dst = AP(s2s, (fl * 4 + q0) * 128 * 512, [(512, 128), (1, 512)])
                getattr(nc, eng or FLUSH_ENG[fl % 2]).dma_start(
                    dst, st[:, q0 : q0 + nq, :] if nq < 4 else st[:]
                )

            # main loop: 32 groups x 8 contiguous rows (8m + 4j + i)
            sts = {}
            for m in range(32):
                ft = ftp.tile([128, 2048], f16)
                # partition p = 64j + c (j = row quad), free = (i, col):
                # element = feat[c, 8m + 4j + i, col]  (4 contiguous rows/half)
                src = AP(feat, 8 * m * W, [(4 * W, 2), (H * W, C), (1, 4 * W)])
                if m == 0:
                    # split the first load across two engines so matmul 0
                    # starts ~1us earlier
                    srcA = AP(feat, 0, [(4 * W, 2), (H * W, C), (1, 2 * W)])
                    srcB = AP(feat, 2 * W, [(4 * W, 2), (H * W, C), (1, 2 * W)])
                    nc.scalar.dma_start(ft[:, 0:1024], srcA)
                    nc.gpsimd.dma_start(ft[:, 1024:2048], srcB)
                elif m == 28:
                    nc.sync.dma_start(ft[:], src)
                elif m == 31:
                    nc.scalar.dma_start(ft[:], src)
                else:
                    getattr(nc, FT_ENG[m % 3]).dma_start(ft[:], src)

                ps = psp.tile([128, 512], f32)
                for i in range(4):
                    nc.tensor.matmul(
                        ps[32 * i : 32 * i + 32, :],
                        lhsT=wt[:],
                        rhs=ft[:, 512 * i : 512 * (i + 1)],
                        start=True,
                        stop=True,
                        tile_position=(0, 32 * i),
                    )

                if m % 4 == 0:
                    sts[m // 4] = stp.tile([128, 4, 512], f16, name="st")
                q = m % 4
                # cast PSUM -> SBUF fp16; last groups alternate DVE/ACT so no
                # copy backlog trails the final ft load
                if m >= 29 and m % 2 == 1:
                    nc.scalar.copy(sts[m // 4][:, q, :], ps[:])
                else:
                    nc.vector.tensor_copy(sts[m // 4][:, q, :], ps[:])

                # flushes 0..5 half-lagged in-loop; end-game handled below
                if m in (7, 11, 15, 19, 23, 26):
                    fl = (m - 7) // 4 if m < 26 else 5
                    emit_flush(fl, sts)
                if m == 29:
                    # flush 6 + prefix doubling: unblocks tap group A
                    emit_flush(6, sts, eng="sync")
                    _dbl_rows(nc, AP, s2s, s2x, 0, 3584, "scalar")
                if m == 30:
                    # group-28 flush + its doubling: unblocks tap group B
                    emit_flush(7, sts, q0=0, nq=1, eng="sync")
                    _dbl_rows(nc, AP, s2s, s2x, 3584, 128, "sync")
                if m == 31:
                    emit_flush(7, sts, q0=1, nq=1, eng="sync")

            # final per-group flushes + suffix doubling (tap group C)
            emit_flush(7, sts, q0=2, nq=1, eng="sync")
            emit_flush(7, sts, q0=3, nq=1, eng="scalar")
            _dbl_rows(nc, AP, s2s, s2x, 3712, 384, "scalar")

            # tap-group gathers: A needs only rows < 3584 (flushes 0-6),
            # B rows < 3712, C everything.  Prefix-shaped in_ APs keep each
            # gather's dependency to its own prefix doubling.
            gas = []
            for g, (i0, nk, rlim) in enumerate(TAP_GROUPS):
                ga = gap.tile([128, nk, 2, 512], f16, name=f"gag{g}")
                nc.gpsimd.indirect_dma_start(
                    out=ga[:].opt(keep_dims={0}),
                    out_offset=None,
                    in_=AP(s2x, 0, [(512, rlim * 2), (1, 512)]),
                    in_offset=IndirectOffsetOnAxis(
                        ap=offs_t[:, i0 : i0 + nk, :], axis=1
                    ),
                )
                gas.append(ga)

            o = outp.tile([128, 2, 512], f16)
            u = outp.tile([128, 2, 512], f16)
            # group A tree (4 adds), then += B pair, then += C pair
            nc.vector.tensor_add(o[:], gas[0][:, 0], gas[0][:, 1])
            nc.vector.tensor_add(u[:], gas[0][:, 2], gas[0][:, 3])
            nc.vector.tensor_add(o[:], o[:], u[:])
            nc.vector.tensor_add(o[:], o[:], gas[0][:, 4])
            nc.vector.tensor_add(u[:], gas[1][:, 0], gas[1][:, 1])
            nc.vector.tensor_add(o[:], o[:], u[:])
            nc.vector.tensor_add(u[:], gas[2][:, 0], gas[2][:, 1])
            nc.vector.tensor_add(o[:], o[:], u[:])

            # column-double + cast to f32 (both on DVE; ACT is mid-DMA)
            o2 = outp.tile([128, 2, 512, 2], f32)
            nc.vector.tensor_copy(o2[:, :, :, 0], o[:])
            nc.vector.tensor_copy(o2[:, :, :, 1], o[:])

            # upsampled rows 4p + 2hd + a: even rows from SP, odd from ACT,
            # concurrently (both read o2).
            dst = AP(out, 0, [(4 * 2 * W, 128), (2 * 2 * W, 2), (1, 2 * W)])
            nc.sync.dma_start(dst, o2[:].opt(keep_dims={0}))
            dst2 = AP(out, 2 * W, [(4 * 2 * W, 128), (2 * 2 * W, 2), (1, 2 * W)])
            nc.scalar.dma_start(dst2, o2[:].opt(keep_dims={0}))

    if split_waits:
        _split_multi_waits(nc, mybir)
    _prog_cache[key] = nc
    return nc


def _structured(gi, gj):
    if not all(np.array_equal(gi[:, :, k], np.broadcast_to(gi[:, :1, k], (H, W))) for k in range(TAPS)):
        return False
    d = (gj - np.arange(W, dtype=np.int64)[None, :, None]) % W
    if not all(np.array_equal(d[:, :, k], np.broadcast_to(d[:, :1, k], (H, W))) for k in range(TAPS)):
        return False
    # per-tap-group source-row bounds assumed by the gather pipeline
    r = gi[:, 0, :]
    for (i0, nk, _), mx in zip(TAP_GROUPS, TAP_MAXROW):
        if max(int(r[:, TAP_PERM[i0 + t]].max()) for t in range(nk)) > mx:
            return False
    return True


def _host_fallback(feature, weight, gi, gj):
    # correct-but-slow path for arbitrary (non roll-structured) index maps
    wflat = weight.reshape(1, C, TAPS).astype(np.float32)
    outc = np.zeros((B, H, W), np.float32)
    for k in range(TAPS):
        xk = feature[:, :, gi[:, :, k], gj[:, :, k]]
        outc += np.einsum("bchw,c->bhw", xk, wflat[0, :, k])
    up = np.repeat(np.repeat(outc, 2, axis=1), 2, axis=2)
    return up[:, None].astype(np.float32)


def _make_device_inputs(weight, gi, gj):
    # block-diag stationary [128, 32]: wt[64*j + c, 9*j + k] = w[c,k]
    w9 = np.asarray(weight, np.float32).reshape(C, TAPS)
    wbd = np.zeros((128, 32), np.float16)
    for j in range(2):
        wbd[64 * j : 64 * j + 64, 9 * j : 9 * j + 9] = w9

    r = gi[:, 0, :].astype(np.int64)  # [H, 9]
    d = gj[:, 0, :].astype(np.int64) % W  # shift per (h, k)

    # scratch row id for source row r = 8m + 4j + i:
    # slot = 32i + 9j + k, row = m*128 + slot  (m = r//8)
    mm = r // 8
    rem = r % 8
    j2 = rem // 4
    i4 = rem % 4
    row_id = (mm * 128 + 32 * i4 + 9 * j2) + np.arange(TAPS)[None, :]
    off_hk = row_id * ROWLEN + d  # [H, 9]

    # offs[p, idx, hd] for output row h = 2p + hd, taps in TAP_PERM order
    offs = np.zeros((128, TAPS, 2), np.int32)
    for hd in range(2):
        offs[:, :, hd] = off_hk[2 * np.arange(128) + hd, :][:, list(TAP_PERM)]
    return wbd, offs


def _run_device(feature16, wbd, offs, trace=False, trace_kwargs=None):
    from concourse.bass_utils import run_bass_kernel_spmd

    nc = _build_program()
    in_maps = [
        {"feat16": np.ascontiguousarray(feature16[b]), "wbd": wbd, "offs": offs}
        for b in range(B)
    ]
    kw = {}
    if trace:
        kw["trace"] = True
        if trace_kwargs:
            kw.update(trace_kwargs)
    return run_bass_kernel_spmd(nc, in_maps, list(range(NCORES)), **kw)


def kernel(feature, weight, gi, gj):
    feature = np.asarray(feature, dtype=np.float32)
    weight = np.asarray(weight, dtype=np.float32)
    gi = np.asarray(gi)
    gj = np.asarray(gj)

    if not _structured(gi, gj):
        return _host_fallback(feature, weight, gi, gj)

    wbd, offs = _make_device_inputs(weight, gi, gj)
    feature16 = feature.astype(np.float16)
    res = _run_device(feature16, wbd, offs)
    out = np.stack([res.results[b]["out"] for b in range(B)])
    return out[:, None].astype(np.float32)
